# revision 14
# baseline (speedup 1.0000x reference)
"""MoD router kernel for 8 Trainium2 NeuronCores.

Full inputs: x [4, 8192, 1024] f32, w_router [1024] f32, w_block [1024, 1024] f32.
out[b, l] = gelu_tanh(x[b, l] @ w_block) if l in topk(x[b] @ w_router, k=6144)
            else x[b, l]
(top-k membership is all that matters: the reference scatters processed rows
back to their own positions.)

Sharding: core c <- batch row c//2, contiguous half c%2 of L (4096 tokens).
Per core:
  - front-loaded streaming of x (8x 2MiB DMAs), x stays resident in SBUF
  - scores = x @ w_router on DVE (f32; preserves the exact top-k set)
  - cast x -> bf16 (DVE), PE-transpose (bf16, 1cyc/row), 16 bf16 matmuls vs
    w_block (K=1024, fp32 PSUM), tanh-gelu on ACT, bulk-write gelu(xW) for
    ALL tokens
  - pairwise AllGather of scores (16KB) -> full-row scores
  - 16-ary threshold search, 9 rounds: one broadcast-compare [P,15,64] +
    reduce per round, cross-partition count via a tiny PE matmul, fused
    interval-select via tensor_scalar accum_out. Grid points are dyadic and
    recomputed bit-identically, so the final lo is exactly the k-th largest.
  - fixup: per-tile indirect scatter overwrites pass-through rows with the
    resident f32 x rows (selected rows get OOB offsets -> skipped); each
    scatter depends only on its tile's store, so the tail overlaps.
"""
import sys

if "/opt/trn_rl_repo" not in sys.path:
    sys.path.insert(0, "/opt/trn_rl_repo")

import os
from contextlib import ExitStack

import numpy as np

import concourse.bass as bass
import concourse.tile as tile
from concourse import bacc, mybir
from concourse.bass_utils import run_bass_kernel_spmd
from concourse.masks import make_identity

dt = mybir.dt
AF = mybir.ActivationFunctionType
ALU = mybir.AluOpType

P = 128
B, L, D = 4, 8192, 1024
TLOC = L // 2          # tokens per core
NT = TLOC // P         # 32 t-tiles per core
DC = D // P            # 8 contraction chunks
K_SEL = int(L * 0.75)  # 6144
N_ROUNDS = 9           # 16^-9 * 128 = 1.9e-9 resolution
SCORE_BOUND = 64.0
LOAD_CHUNK = 4         # t-tiles per load DMA (2 MiB)

_cached = {}


def build_kernel():
    nc = bacc.Bacc("TRN2", target_bir_lowering=False, debug=False, num_devices=8)
    x_d = nc.dram_tensor("x", [TLOC, D], dt.float32, kind="ExternalInput")
    wr_d = nc.dram_tensor("w_router", [D], dt.float32, kind="ExternalInput")
    wb_d = nc.dram_tensor("w_block", [D, D], dt.float32, kind="ExternalInput")
    out_d = nc.dram_tensor("out", [TLOC, D], dt.float32, kind="ExternalOutput")
    sc_in = nc.dram_tensor("sc_in", [TLOC], dt.float32, kind="Internal")
    sc_out = nc.dram_tensor("sc_out", [L], dt.float32, kind="Internal")
    dbg = os.environ.get("KERNEL_DEBUG") == "1"
    if dbg:
        dbg_lo = nc.dram_tensor("dbg_lo", [P, N_ROUNDS], dt.float32,
                                kind="ExternalOutput")
        dbg_msel = nc.dram_tensor("dbg_msel", [P, NT], dt.float32,
                                  kind="ExternalOutput")

    with tile.TileContext(nc) as tc, ExitStack() as ctx:
        const = ctx.enter_context(tc.tile_pool(name="const", bufs=1))
        xpool = ctx.enter_context(tc.tile_pool(name="xn", bufs=1))
        wpool = ctx.enter_context(tc.tile_pool(name="wb", bufs=1))
        xbfp = ctx.enter_context(tc.tile_pool(name="xbf", bufs=4))
        xtp = ctx.enter_context(tc.tile_pool(name="xt", bufs=4))
        yp = ctx.enter_context(tc.tile_pool(name="y", bufs=2))
        smalls = ctx.enter_context(tc.tile_pool(name="smalls", bufs=1))
        psx = ctx.enter_context(tc.tile_pool(name="psx", bufs=2, space="PSUM"))
        psy = ctx.enter_context(tc.tile_pool(name="psy", bufs=3, space="PSUM"))

        # ---- constants ----
        ident = const.tile([P, P], dt.bfloat16)
        make_identity(nc, ident[:])
        ones_pp = const.tile([P, P], dt.float32)
        nc.vector.memset(ones_pp[:], 1.0)
        ones_row = const.tile([1, P], dt.float32)
        nc.vector.memset(ones_row[:], 1.0)

        # w_block -> bf16 SBUF [p, dc, e]. HWDGE f32 staging + ACT cast:
        # keeps gpsimd (SWDGE) free so the collective trigger fires early.
        w_sb = wpool.tile([P, DC, D], dt.bfloat16)
        for c in range(DC):
            wstage = wpool.tile([P, D], dt.float32, tag="wstage", bufs=2)
            nc.sync.dma_start(wstage[:], wb_d.ap()[c * P:(c + 1) * P, :])
            nc.scalar.copy(w_sb[:, c, :], wstage[:])

        wr_sb = const.tile([1, D], dt.float32)
        nc.sync.dma_start(wr_sb[:], wr_d.ap())
        # broadcast w_router over all partitions via K=1 matmuls
        w_rep = const.tile([P, D], dt.float32)
        for h in range(2):
            sl = slice(h * 512, (h + 1) * 512)
            pm = psy.tile([P, D], dt.float32, tag="psy")
            nc.tensor.matmul(pm[:, :512], ones_row[:], wr_sb[:, sl],
                             start=True, stop=True)
            nc.vector.tensor_copy(w_rep[:, sl], pm[:, :512])

        # ---- score / search tiles ----
        scores_loc = smalls.tile([P, NT], dt.float32)
        scores_full = smalls.tile([P, 2 * NT], dt.float32)
        ge3 = smalls.tile([P, 15, 2 * NT], dt.float32)
        cnts = smalls.tile([P, 15], dt.float32)
        gk = smalls.tile([P, 15], dt.float32)
        tcand = smalls.tile([P, 15], dt.float32)
        jrow_i = smalls.tile([P, 15], dt.int32)
        jrow = smalls.tile([P, 15], dt.float32)
        lo = smalls.tile([P, 1], dt.float32)
        w16t = smalls.tile([P, 1], dt.float32)
        m = smalls.tile([P, 1], dt.float32)
        msel = smalls.tile([P, NT], dt.float32)
        pcol_i = smalls.tile([P, 1], dt.int32)
        pcol = smalls.tile([P, 1], dt.float32)
        offs_f = smalls.tile([P, NT], dt.float32)
        offs = smalls.tile([P, NT], dt.int32)
        tmp = smalls.tile([P, D], dt.float32)

        # ---- front-loaded streaming + scores ----
        xn_all = xpool.tile([P, NT, D], dt.float32)
        for j in range(NT // LOAD_CHUNK):
            a = j * LOAD_CHUNK
            with nc.named_scope("load"):
                nc.sync.dma_start(
                    xn_all[:, a:a + LOAD_CHUNK, :],
                    x_d.ap()[a * P:(a + LOAD_CHUNK) * P, :].rearrange(
                        "(c p) d -> p c d", p=P))
            with nc.named_scope("scores"), tc.high_priority():
                for i in range(a, a + LOAD_CHUNK):
                    nc.vector.tensor_tensor(out=tmp[:], in0=xn_all[:, i, :],
                                            in1=w_rep[:], op=ALU.mult)
                    nc.vector.reduce_sum(scores_loc[:, i:i + 1], tmp[:],
                                         axis=mybir.AxisListType.X)

        # ---- main compute loop ----
        store_insts = []
        for i in range(NT):
            with nc.named_scope("cast"):
                xbf = xbfp.tile([P, D], dt.bfloat16, tag="xbf")
                nc.scalar.copy(xbf[:], xn_all[:, i, :])
            xt = xtp.tile([P, DC, P], dt.bfloat16, tag="xt")
            px = psx.tile([P, DC, P], dt.bfloat16, tag="psx")
            with nc.named_scope("xpose"):
                for c in range(DC):
                    nc.tensor.transpose(px[:, c, :], xbf[:, c * P:(c + 1) * P],
                                        ident[:])
                nc.vector.tensor_copy(out=xt[:], in_=px[:])
            y = yp.tile([P, D], dt.float32, tag="y")
            py = psy.tile([P, D], dt.float32, tag="psy")
            with nc.named_scope("gemm"):
                for h in range(2):
                    for c in range(DC):
                        nc.tensor.matmul(
                            py[:, h * 512:(h + 1) * 512], xt[:, c, :],
                            w_sb[:, c, h * 512:(h + 1) * 512],
                            start=(c == 0), stop=(c == DC - 1))
            with nc.named_scope("gelu"):
                nc.scalar.activation(y[:], py[:], AF.Gelu_apprx_tanh)
            with nc.named_scope("store"):
                st = nc.sync.dma_start(out_d.ap()[i * P:(i + 1) * P, :], y[:])
            store_insts.append(st)

        # ---- threshold side-chain (high priority so it never starves) ----
        with tc.high_priority():
            with nc.named_scope("coll"):
                nc.sync.dma_start(sc_in.ap(), scores_loc[:])
                nc.gpsimd.collective_compute(
                    "AllGather", ALU.bypass,
                    ins=[sc_in.ap()], outs=[sc_out.ap()],
                    replica_groups=[[0, 1], [2, 3], [4, 5], [6, 7]])
                nc.sync.dma_start(scores_full[:], sc_out.ap())
            with nc.named_scope("search"):
                # jrow = 1..15 replicated on every partition
                nc.gpsimd.iota(jrow_i[:], pattern=[[1, 15]], base=1,
                               channel_multiplier=0)
                nc.vector.tensor_copy(out=jrow[:], in_=jrow_i[:])
                nc.vector.memset(lo[:], -SCORE_BOUND)
                nc.vector.memset(w16t[:], 2.0 * SCORE_BOUND / 16.0)
                sc_b = scores_full[:].rearrange("p (a x) -> p a x", a=1) \
                    .to_broadcast([P, 15, 2 * NT])
                t_b = tcand[:].rearrange("p (j x) -> p j x", x=1) \
                    .to_broadcast([P, 15, 2 * NT])
                for r in range(N_ROUNDS):
                    # tcand[:, j] = lo + (j+1)*w16  (dyadic, exact fp32)
                    nc.vector.tensor_scalar(out=tcand[:], in0=jrow[:],
                                            scalar1=w16t[:], scalar2=lo[:],
                                            op0=ALU.mult, op1=ALU.add)
                    nc.vector.tensor_tensor(out=ge3[:], in0=sc_b, in1=t_b,
                                            op=ALU.is_ge)
                    nc.vector.reduce_sum(cnts[:], ge3[:],
                                         axis=mybir.AxisListType.X)
                    pc = psy.tile([P, D], dt.float32, tag="psy")
                    nc.tensor.matmul(pc[:, :15], ones_pp[:], cnts[:],
                                     start=True, stop=True)
                    # gk = (count >= k); m = #intervals passed (row-sum)
                    nc.vector.tensor_scalar(out=gk[:], in0=pc[:, :15],
                                            scalar1=float(K_SEL), scalar2=None,
                                            op0=ALU.is_ge)
                    nc.vector.reduce_sum(m[:], gk[:],
                                         axis=mybir.AxisListType.X)
                    # lo += m*w16 (bit-identical to the compared grid point)
                    nc.vector.tensor_scalar(out=lo[:], in0=m[:],
                                            scalar1=w16t[:], scalar2=lo[:],
                                            op0=ALU.mult, op1=ALU.add)
                    nc.vector.tensor_scalar_mul(w16t[:], w16t[:], 1.0 / 16.0)
                    if dbg:
                        nc.sync.dma_start(dbg_lo.ap()[:, r:r + 1], lo[:])
            with nc.named_scope("mask"):
                # selected = score >= thr(=lo); offs = p + sel*2^30 (per-tile)
                nc.vector.tensor_scalar(out=msel[:], in0=scores_loc[:],
                                        scalar1=lo[:], scalar2=None,
                                        op0=ALU.is_ge)
                nc.gpsimd.iota(pcol_i[:], pattern=[[0, 1]], base=0,
                               channel_multiplier=1)
                nc.vector.tensor_copy(out=pcol[:], in_=pcol_i[:])
                nc.vector.tensor_scalar(out=offs_f[:], in0=msel[:],
                                        scalar1=float(2 ** 30),
                                        scalar2=pcol[:],
                                        op0=ALU.mult, op1=ALU.add)
                nc.vector.tensor_copy(out=offs[:], in_=offs_f[:])
                if dbg:
                    nc.sync.dma_start(dbg_msel.ap(), msel[:])

        # ---- fixup: overwrite pass-through rows with resident x rows ----
        with nc.named_scope("fixup"):
            for i in range(NT):
                sl = out_d.ap()[i * P:(i + 1) * P, :]
                sl_rel = bass.AP(tensor=sl.tensor, offset=0, ap=sl.ap,
                                 dep_tracking_offset=i * P * D)
                fx = nc.gpsimd.indirect_dma_start(
                    out=sl_rel,
                    out_offset=bass.IndirectOffsetOnAxis(ap=offs[:, i:i + 1],
                                                         axis=0),
                    in_=xn_all[:, i, :],
                    in_offset=None,
                    element_offset=i * P * D,
                    bounds_check=P - 1,
                    oob_is_err=False,
                )
                tile.add_dep_helper(fx.ins, store_insts[i].ins,
                                    reason="fixup scatter after bulk y store")

    nc.compile()
    return nc


def _get_nc():
    if "nc" not in _cached:
        _cached["nc"] = build_kernel()
    return _cached["nc"]


def run(x, w_router, w_block, trace=False, trace_kwargs=None):
    nc = _get_nc()
    x = np.ascontiguousarray(x, dtype=np.float32)
    w_router = np.ascontiguousarray(w_router, dtype=np.float32)
    w_block = np.ascontiguousarray(w_block, dtype=np.float32)
    in_maps = []
    for c in range(8):
        b, h = c // 2, c % 2
        in_maps.append({
            "x": x[b, h * TLOC:(h + 1) * TLOC, :],
            "w_router": w_router,
            "w_block": w_block,
        })
    res = run_bass_kernel_spmd(nc, in_maps, core_ids=list(range(8)),
                               trace=trace, **(trace_kwargs or {}))
    out = np.empty((B, L, D), dtype=np.float32)
    for c in range(8):
        b, h = c // 2, c % 2
        out[b, h * TLOC:(h + 1) * TLOC, :] = res.results[c]["out"]
    return out, res


def kernel(x, w_router, w_block):
    out, _ = run(x, w_router, w_block, trace=False)
    return out


# revision 15
# speedup vs baseline: 1.1995x; 1.1995x over previous
"""MoD router kernel for 8 Trainium2 NeuronCores.

Full inputs: x [4, 8192, 1024] f32, w_router [1024] f32, w_block [1024, 1024] f32.
out[b, l] = gelu_tanh(x[b, l] @ w_block) if l in topk(x[b] @ w_router, k=6144)
            else x[b, l]
(top-k membership is all that matters: the reference scatters processed rows
back to their own positions.)

Sharding: core c <- batch row c//2, contiguous half c%2 of L (4096 tokens).
Per core:
  - front-loaded streaming of x (8x 2MiB DMAs), x stays resident in SBUF
  - scores = x @ w_router on DVE (f32; preserves the exact top-k set)
  - cast x -> bf16 (DVE), PE-transpose (bf16, 1cyc/row), 16 bf16 matmuls vs
    w_block (K=1024, fp32 PSUM), tanh-gelu on ACT, bulk-write gelu(xW) for
    ALL tokens
  - pairwise AllGather of scores (16KB) -> full-row scores
  - 16-ary threshold search, 9 rounds: one broadcast-compare [P,15,64] +
    reduce per round, cross-partition count via a tiny PE matmul, fused
    interval-select via tensor_scalar accum_out. Grid points are dyadic and
    recomputed bit-identically, so the final lo is exactly the k-th largest.
  - fixup: per-tile indirect scatter overwrites pass-through rows with the
    resident f32 x rows (selected rows get OOB offsets -> skipped); each
    scatter depends only on its tile's store, so the tail overlaps.
"""
import sys

if "/opt/trn_rl_repo" not in sys.path:
    sys.path.insert(0, "/opt/trn_rl_repo")

import os
from contextlib import ExitStack

import numpy as np

import concourse.bass as bass
import concourse.tile as tile
from concourse import bacc, mybir
from concourse.bass_utils import run_bass_kernel_spmd
from concourse.masks import make_identity
from concourse import bass_isa

dt = mybir.dt
AF = mybir.ActivationFunctionType
ALU = mybir.AluOpType

P = 128
B, L, D = 4, 8192, 1024
TLOC = L // 2          # tokens per core
NT = TLOC // P         # 32 t-tiles per core
DC = D // P            # 8 contraction chunks
K_SEL = int(L * 0.75)  # 6144
N_ROUNDS = 9           # 16^-9 * 128 = 1.9e-9 resolution
SCORE_BOUND = 64.0
LOAD_CHUNK = 4         # t-tiles per load DMA (2 MiB)

_cached = {}


def build_kernel():
    nc = bacc.Bacc("TRN2", target_bir_lowering=False, debug=False, num_devices=8)
    x_d = nc.dram_tensor("x", [TLOC, D], dt.float32, kind="ExternalInput")
    wr_d = nc.dram_tensor("w_router", [D], dt.float32, kind="ExternalInput")
    wb_d = nc.dram_tensor("w_block", [D, D], dt.float32, kind="ExternalInput")
    out_d = nc.dram_tensor("out", [TLOC, D], dt.float32, kind="ExternalOutput")
    sc_in = nc.dram_tensor("sc_in", [TLOC], dt.float32, kind="Internal")
    sc_out = nc.dram_tensor("sc_out", [L], dt.float32, kind="Internal")
    dbg = os.environ.get("KERNEL_DEBUG") == "1"
    if dbg:
        dbg_lo = nc.dram_tensor("dbg_lo", [P, N_ROUNDS], dt.float32,
                                kind="ExternalOutput")
        dbg_msel = nc.dram_tensor("dbg_msel", [P, NT], dt.float32,
                                  kind="ExternalOutput")

    with tile.TileContext(nc) as tc, ExitStack() as ctx:
        const = ctx.enter_context(tc.tile_pool(name="const", bufs=1))
        xpool = ctx.enter_context(tc.tile_pool(name="xn", bufs=1))
        wpool = ctx.enter_context(tc.tile_pool(name="wb", bufs=1))
        xbfp = ctx.enter_context(tc.tile_pool(name="xbf", bufs=4))
        xtp = ctx.enter_context(tc.tile_pool(name="xt", bufs=4))
        yp = ctx.enter_context(tc.tile_pool(name="y", bufs=2))
        smalls = ctx.enter_context(tc.tile_pool(name="smalls", bufs=1))
        psx = ctx.enter_context(tc.tile_pool(name="psx", bufs=2, space="PSUM"))
        psy = ctx.enter_context(tc.tile_pool(name="psy", bufs=3, space="PSUM"))

        # ---- constants ----
        ident = const.tile([P, P], dt.bfloat16)
        make_identity(nc, ident[:])
        ones_pp = const.tile([P, P], dt.float32)
        nc.vector.memset(ones_pp[:], 1.0)
        ones_row = const.tile([1, P], dt.float32)
        nc.vector.memset(ones_row[:], 1.0)

        # w_block -> bf16 SBUF [p, dc, e]. HWDGE f32 staging + ACT cast:
        # keeps gpsimd (SWDGE) free so the collective trigger fires early.
        w_sb = wpool.tile([P, DC, D], dt.bfloat16)
        for c in range(DC):
            wstage = wpool.tile([P, D], dt.float32, tag="wstage", bufs=2)
            nc.sync.dma_start(wstage[:], wb_d.ap()[c * P:(c + 1) * P, :])
            nc.scalar.copy(w_sb[:, c, :], wstage[:])

        wr_sb = const.tile([1, D], dt.float32)
        nc.sync.dma_start(wr_sb[:], wr_d.ap())
        # broadcast w_router over all partitions via K=1 matmuls
        w_rep = const.tile([P, D], dt.float32)
        for h in range(2):
            sl = slice(h * 512, (h + 1) * 512)
            pm = psy.tile([P, D], dt.float32, tag="psy")
            nc.tensor.matmul(pm[:, :512], ones_row[:], wr_sb[:, sl],
                             start=True, stop=True)
            nc.vector.tensor_copy(w_rep[:, sl], pm[:, :512])

        # ---- score / search tiles ----
        scores_loc = smalls.tile([P, NT], dt.float32)
        scores_full = smalls.tile([P, 2 * NT], dt.float32)
        ge3 = smalls.tile([P, 15, 2 * NT], dt.float32)
        cnts = smalls.tile([P, 15], dt.float32)
        gk = smalls.tile([P, 15], dt.float32)
        tcand = smalls.tile([P, 15], dt.float32)
        jrow_i = smalls.tile([P, 15], dt.int32)
        jrow = smalls.tile([P, 15], dt.float32)
        lo = smalls.tile([P, 1], dt.float32)
        w16t = smalls.tile([P, 1], dt.float32)
        m = smalls.tile([P, 1], dt.float32)
        msel = smalls.tile([P, NT], dt.float32)
        pcol_i = smalls.tile([P, 1], dt.int32)
        pcol = smalls.tile([P, 1], dt.float32)
        offs_f = smalls.tile([P, NT], dt.float32)
        offs = smalls.tile([P, NT], dt.int32)
        tmp = smalls.tile([P, D], dt.float32)
        tmpg = smalls.tile([P, D], dt.float32)
        cnts_red = smalls.tile([P, 15], dt.float32)

        # ---- front-loaded streaming + scores ----
        xn_all = xpool.tile([P, NT, D], dt.float32)
        for j in range(NT // LOAD_CHUNK):
            a = j * LOAD_CHUNK
            with nc.named_scope("load"):
                nc.sync.dma_start(
                    xn_all[:, a:a + LOAD_CHUNK, :],
                    x_d.ap()[a * P:(a + LOAD_CHUNK) * P, :].rearrange(
                        "(c p) d -> p c d", p=P))
            with nc.named_scope("scores"), tc.high_priority():
                for i in range(a, a + LOAD_CHUNK):
                    # alternate the mul between gpsimd and DVE so neither
                    # engine serializes the whole score pass
                    if i % 2 == 0:
                        nc.vector.tensor_tensor(out=tmp[:],
                                                in0=xn_all[:, i, :],
                                                in1=w_rep[:], op=ALU.mult)
                        nc.vector.reduce_sum(scores_loc[:, i:i + 1], tmp[:],
                                             axis=mybir.AxisListType.X)
                    else:
                        nc.gpsimd.tensor_tensor(out=tmpg[:],
                                                in0=xn_all[:, i, :],
                                                in1=w_rep[:], op=ALU.mult)
                        nc.vector.reduce_sum(scores_loc[:, i:i + 1], tmpg[:],
                                             axis=mybir.AxisListType.X)

        # ---- main compute loop ----
        store_insts = []
        for i in range(NT):
            with nc.named_scope("cast"):
                xbf = xbfp.tile([P, D], dt.bfloat16, tag="xbf")
                nc.scalar.copy(xbf[:], xn_all[:, i, :])
            xt = xtp.tile([P, DC, P], dt.bfloat16, tag="xt")
            px = psx.tile([P, DC, P], dt.bfloat16, tag="psx")
            with nc.named_scope("xpose"):
                for c in range(DC):
                    nc.tensor.transpose(px[:, c, :], xbf[:, c * P:(c + 1) * P],
                                        ident[:])
                nc.vector.tensor_copy(out=xt[:], in_=px[:])
            y = yp.tile([P, D], dt.float32, tag="y")
            py = psy.tile([P, D], dt.float32, tag="psy")
            with nc.named_scope("gemm"):
                for h in range(2):
                    for c in range(DC):
                        nc.tensor.matmul(
                            py[:, h * 512:(h + 1) * 512], xt[:, c, :],
                            w_sb[:, c, h * 512:(h + 1) * 512],
                            start=(c == 0), stop=(c == DC - 1))
            with nc.named_scope("gelu"):
                nc.scalar.activation(y[:], py[:], AF.Gelu_apprx_tanh)
            with nc.named_scope("store"):
                st = nc.sync.dma_start(out_d.ap()[i * P:(i + 1) * P, :], y[:])
            store_insts.append(st)

        # ---- threshold side-chain (high priority so it never starves) ----
        with tc.high_priority():
            with nc.named_scope("coll"):
                nc.sync.dma_start(sc_in.ap(), scores_loc[:])
                nc.gpsimd.collective_compute(
                    "AllGather", ALU.bypass,
                    ins=[sc_in.ap()], outs=[sc_out.ap()],
                    replica_groups=[[0, 1], [2, 3], [4, 5], [6, 7]])
                nc.sync.dma_start(scores_full[:], sc_out.ap())
            with nc.named_scope("search"):
                # jrow = 1..15 replicated on every partition
                nc.gpsimd.iota(jrow_i[:], pattern=[[1, 15]], base=1,
                               channel_multiplier=0)
                nc.vector.tensor_copy(out=jrow[:], in_=jrow_i[:])
                nc.vector.memset(lo[:], -SCORE_BOUND)
                nc.vector.memset(w16t[:], 2.0 * SCORE_BOUND / 16.0)
                sc_b = scores_full[:].rearrange("p (a x) -> p a x", a=1) \
                    .to_broadcast([P, 15, 2 * NT])
                t_b = tcand[:].rearrange("p (j x) -> p j x", x=1) \
                    .to_broadcast([P, 15, 2 * NT])
                for r in range(N_ROUNDS):
                    # tcand[:, j] = lo + (j+1)*w16  (dyadic, exact fp32)
                    nc.vector.tensor_scalar(out=tcand[:], in0=jrow[:],
                                            scalar1=w16t[:], scalar2=lo[:],
                                            op0=ALU.mult, op1=ALU.add)
                    nc.vector.tensor_tensor(out=ge3[:], in0=sc_b, in1=t_b,
                                            op=ALU.is_ge)
                    nc.vector.reduce_sum(cnts[:], ge3[:],
                                         axis=mybir.AxisListType.X)
                    nc.gpsimd.partition_all_reduce(
                        cnts_red[:], cnts[:], P, bass_isa.ReduceOp.add)
                    # gk = (count >= k); m = #intervals passed (row-sum)
                    nc.vector.tensor_scalar(out=gk[:], in0=cnts_red[:],
                                            scalar1=float(K_SEL), scalar2=None,
                                            op0=ALU.is_ge)
                    nc.vector.reduce_sum(m[:], gk[:],
                                         axis=mybir.AxisListType.X)
                    # lo += m*w16 (bit-identical to the compared grid point)
                    nc.vector.tensor_scalar(out=lo[:], in0=m[:],
                                            scalar1=w16t[:], scalar2=lo[:],
                                            op0=ALU.mult, op1=ALU.add)
                    nc.vector.tensor_scalar_mul(w16t[:], w16t[:], 1.0 / 16.0)
                    if dbg:
                        nc.sync.dma_start(dbg_lo.ap()[:, r:r + 1], lo[:])
            with nc.named_scope("mask"):
                # selected = score >= thr(=lo); offs = p + sel*2^30 (per-tile)
                nc.vector.tensor_scalar(out=msel[:], in0=scores_loc[:],
                                        scalar1=lo[:], scalar2=None,
                                        op0=ALU.is_ge)
                nc.gpsimd.iota(pcol_i[:], pattern=[[0, 1]], base=0,
                               channel_multiplier=1)
                nc.vector.tensor_copy(out=pcol[:], in_=pcol_i[:])
                nc.vector.tensor_scalar(out=offs_f[:], in0=msel[:],
                                        scalar1=float(2 ** 30),
                                        scalar2=pcol[:],
                                        op0=ALU.mult, op1=ALU.add)
                nc.vector.tensor_copy(out=offs[:], in_=offs_f[:])
                if dbg:
                    nc.sync.dma_start(dbg_msel.ap(), msel[:])

        # ---- fixup: overwrite pass-through rows with resident x rows ----
        with nc.named_scope("fixup"):
            for i in range(NT):
                sl = out_d.ap()[i * P:(i + 1) * P, :]
                sl_rel = bass.AP(tensor=sl.tensor, offset=0, ap=sl.ap,
                                 dep_tracking_offset=i * P * D)
                fx = nc.gpsimd.indirect_dma_start(
                    out=sl_rel,
                    out_offset=bass.IndirectOffsetOnAxis(ap=offs[:, i:i + 1],
                                                         axis=0),
                    in_=xn_all[:, i, :],
                    in_offset=None,
                    element_offset=i * P * D,
                    bounds_check=P - 1,
                    oob_is_err=False,
                )
                tile.add_dep_helper(fx.ins, store_insts[i].ins,
                                    reason="fixup scatter after bulk y store")

    nc.compile()
    return nc


def _get_nc():
    if "nc" not in _cached:
        _cached["nc"] = build_kernel()
    return _cached["nc"]


def run(x, w_router, w_block, trace=False, trace_kwargs=None):
    nc = _get_nc()
    x = np.ascontiguousarray(x, dtype=np.float32)
    w_router = np.ascontiguousarray(w_router, dtype=np.float32)
    w_block = np.ascontiguousarray(w_block, dtype=np.float32)
    in_maps = []
    for c in range(8):
        b, h = c // 2, c % 2
        in_maps.append({
            "x": x[b, h * TLOC:(h + 1) * TLOC, :],
            "w_router": w_router,
            "w_block": w_block,
        })
    res = run_bass_kernel_spmd(nc, in_maps, core_ids=list(range(8)),
                               trace=trace, **(trace_kwargs or {}))
    out = np.empty((B, L, D), dtype=np.float32)
    for c in range(8):
        b, h = c // 2, c % 2
        out[b, h * TLOC:(h + 1) * TLOC, :] = res.results[c]["out"]
    return out, res


def kernel(x, w_router, w_block):
    out, _ = run(x, w_router, w_block, trace=False)
    return out
